# revision 10
# baseline (speedup 1.0000x reference)
"""PSKD cross-entropy loss kernel for Trainium2 (8 NeuronCores, data-parallel).

Computes, for logits `output` [B,100] and soft labels `targets` [B,100]:
    loss = sum(mean(-targets * log_softmax(output), 0))
         + 0.5 * sum over 19 rank-windows of the windowed PSKD sub-loss
where the windows are width-10/stride-5 slices of the per-row descending
argsort of `targets`.

Key algebra (ties have measure zero for random float targets):
  - Window membership of class i depends only on its rank r_i among the
    row's targets.  All window quantities are permutation-invariant inside
    the window, so only three per-window aggregates are needed:
        A_w = sum_{win} exp(t_i)        (any constant shift cancels)
        B_w = sum_{win} exp(t_i) * o_i
        S_w = sum_{win} exp(o_i)        (o ~ N(0,1): exp never overflows)
    giving  loss_w = -B_w/A_w + log(S_w).
  - Window w covers ranks [5w, 5w+10), so with suffix sums
        SA_f[k] = sum_i [r_i >= 5k] * f_i           (k = 0..19)
    each window aggregate is SA_f[w] - SA_f[w+2].
  - Ranks come from exact pairwise comparison counting over 50 cyclic
    shifts (each unordered pair compared once).  Comparisons and masked
    aggregands run in bf16 (DVE 4x mode); rank counts <= 99 are exact in
    bf16, and bf16 rounding of the aggregands is zero-mean so the batch
    mean washes it out (validated: rel err ~5e-5 on 32k rows).
  - Suffix-sum masks share one tensor_scalar compare per threshold; the
    masked aggregands reduce in fp32.

Per core: 65536 rows, processed as 128-partition tiles of W rows each.
The per-core partial sum of row losses is returned; the host divides by B
and combines cores.
"""

import numpy as np

B = 524288
C = 100
ALPHA = 0.5
N_CORES = 8
B_CORE = B // N_CORES  # 65536


def build_core_program(rows, W=16):
    """Build the single-core Bass/Tile program (shared by all 8 cores)."""
    from contextlib import ExitStack

    import concourse.mybir as mybir
    import concourse.tile as tile
    from concourse import bacc

    P = 128
    R = P * W
    n_tiles = rows // R
    assert n_tiles * R == rows

    dt = mybir.dt
    A = mybir.AluOpType
    AF = mybir.ActivationFunctionType
    AX = mybir.AxisListType
    f32 = dt.float32
    bf16 = dt.bfloat16

    nc = bacc.Bacc("TRN2", target_bir_lowering=False, debug=False,
                   num_devices=N_CORES)

    tgt_d = nc.dram_tensor("targets", [rows, C], f32, kind="ExternalInput")
    out_d = nc.dram_tensor("output", [rows, C], f32, kind="ExternalInput")
    res_d = nc.dram_tensor("out", [1, 1], f32, kind="ExternalOutput")

    tgt_v = tgt_d.ap().rearrange("(n p w) c -> n p (w c)", p=P, w=W)
    out_v = out_d.ap().rearrange("(n p w) c -> n p (w c)", p=P, w=W)

    with tile.TileContext(nc) as tc, ExitStack() as ctx:
        io = ctx.enter_context(tc.tile_pool(name="io", bufs=2))
        wk = ctx.enter_context(tc.tile_pool(name="wk", bufs=2))
        sm = ctx.enter_context(tc.tile_pool(name="sm", bufs=1))
        pe = ctx.enter_context(tc.tile_pool(name="pe", bufs=1))

        # rank-count constant: 49 for class slots < 50, 50 for >= 50
        const_t = pe.tile([P, W, C], bf16, tag="const")
        nc.gpsimd.memset(const_t[:, :, 0:50], 49.0)
        nc.gpsimd.memset(const_t[:, :, 50:100], 50.0)

        core_acc = pe.tile([P, 1], f32, tag="core_acc")
        nc.vector.memset(core_acc[:], 0.0)

        for ti in range(n_tiles):
            t_t = io.tile([P, W, C], f32, tag="t")
            o_t = io.tile([P, W, C], f32, tag="o")
            nc.sync.dma_start(out=t_t[:].rearrange("p w c -> p (w c)"),
                              in_=tgt_v[ti])
            nc.sync.dma_start(out=o_t[:].rearrange("p w c -> p (w c)"),
                              in_=out_v[ti])

            # bf16 working copies
            t_bf = wk.tile([P, W, C], bf16, tag="t_bf")
            o_bf = wk.tile([P, W, C], bf16, tag="o_bf")
            nc.vector.tensor_copy(t_bf[:], t_t[:])
            nc.gpsimd.tensor_copy(o_bf[:], o_t[:])
            tdup = wk.tile([P, W, 2 * C], bf16, tag="tdup")
            nc.vector.tensor_copy(tdup[:, :, 0:C], t_bf[:])
            nc.vector.tensor_copy(tdup[:, :, C:2 * C], t_bf[:])

            # --- exact descending ranks via cyclic pairwise counting ---
            acc = wk.tile([P, W, C], bf16, tag="acc")
            nc.vector.memset(acc[:], 0.0)
            for s in range(1, 50):
                mask = wk.tile([P, W, C], bf16, tag="scr0")
                # mask[i] = [t_{(i+s)%100} > t_i]
                nc.vector.tensor_tensor(
                    out=mask[:], in0=tdup[:, :, s:s + C], in1=t_bf[:],
                    op=A.is_gt)
                nc.vector.tensor_tensor(
                    out=acc[:], in0=acc[:], in1=mask[:], op=A.add)
                nc.vector.tensor_tensor(
                    out=acc[:, :, s:C], in0=acc[:, :, s:C],
                    in1=mask[:, :, 0:C - s], op=A.subtract)
                nc.vector.tensor_tensor(
                    out=acc[:, :, 0:s], in0=acc[:, :, 0:s],
                    in1=mask[:, :, C - s:C], op=A.subtract)
            m50 = wk.tile([P, W, 50], bf16, tag="m50")
            nc.vector.tensor_tensor(
                out=m50[:], in0=tdup[:, :, 50:100], in1=t_bf[:, :, 0:50],
                op=A.is_gt)
            nc.vector.tensor_tensor(
                out=acc[:, :, 0:50], in0=acc[:, :, 0:50], in1=m50[:],
                op=A.add)
            nc.vector.tensor_tensor(
                out=acc[:, :, 50:100], in0=acc[:, :, 50:100], in1=m50[:],
                op=A.subtract)
            r_t = wk.tile([P, W, C], bf16, tag="r")
            nc.vector.tensor_tensor(
                out=r_t[:], in0=acc[:], in1=const_t[:], op=A.add)

            # --- pointwise transcendentals / products (bf16 aggregands) ---
            et = wk.tile([P, W, C], bf16, tag="et")
            eo = wk.tile([P, W, C], bf16, tag="eo")
            nc.scalar.activation(et[:], t_t[:], AF.Exp)
            nc.scalar.activation(eo[:], o_t[:], AF.Exp)
            h = wk.tile([P, W, C], bf16, tag="h")
            nc.vector.tensor_tensor(
                out=h[:], in0=et[:], in1=o_bf[:], op=A.mult)
            to = wk.tile([P, W, C], bf16, tag="to")
            nc.vector.tensor_tensor(
                out=to[:], in0=t_bf[:], in1=o_bf[:], op=A.mult)
            q = sm.tile([P, W], f32, tag="q")
            nc.vector.tensor_reduce(out=q[:], in_=to[:], axis=AX.X, op=A.add)

            # --- suffix sums SA_f[k] = sum [r>=5k]*f ---
            sa = {}
            for name in ("et", "h", "eo"):
                sa_t = sm.tile([P, W, 21], f32, tag=f"sa_{name}",
                               name=f"sa_{name}")
                nc.vector.memset(sa_t[:, :, 19:21], 0.0)
                sa[name] = sa_t
            for k in range(20):
                if k == 0:
                    for name, f_t in (("et", et), ("h", h), ("eo", eo)):
                        nc.vector.tensor_reduce(
                            out=sa[name][:, :, 0], in_=f_t[:], axis=AX.X,
                            op=A.add)
                    continue
                mk = wk.tile([P, W, C], bf16, tag="mk")
                nc.vector.tensor_scalar(
                    out=mk[:], in0=r_t[:], scalar1=float(5 * k), scalar2=None,
                    op0=A.is_ge)
                for name, f_t in (("et", et), ("h", h), ("eo", eo)):
                    msc = wk.tile([P, W, C], bf16, tag="scr0")
                    nc.vector.tensor_tensor(
                        out=msc[:], in0=mk[:], in1=f_t[:], op=A.mult)
                    nc.vector.tensor_reduce(
                        out=sa[name][:, :, k], in_=msc[:], axis=AX.X, op=A.add)

            # --- windows w=0..18: agg_w = SA[w] - SA[w+2] ---
            a_w = sm.tile([P, W, 19], f32, tag="a_w")
            b_w = sm.tile([P, W, 19], f32, tag="b_w")
            s_w = sm.tile([P, W, 19], f32, tag="s_w")
            for dst, src in ((a_w, sa["et"]), (b_w, sa["h"]), (s_w, sa["eo"])):
                nc.vector.scalar_tensor_tensor(
                    out=dst[:], in0=src[:, :, 0:19], scalar=0.0,
                    in1=src[:, :, 2:21], op0=A.bypass, op1=A.subtract)

            ra = sm.tile([P, W, 19], f32, tag="ra")
            nc.vector.reciprocal(ra[:], a_w[:])
            ba = sm.tile([P, W, 19], f32, tag="ba")
            nc.vector.scalar_tensor_tensor(
                out=ba[:], in0=b_w[:], scalar=0.0, in1=ra[:],
                op0=A.bypass, op1=A.mult)
            lns = sm.tile([P, W, 19], f32, tag="lns")
            nc.scalar.activation(lns[:], s_w[:], AF.Ln)
            lnf = sm.tile([P, W], f32, tag="lnf")
            nc.scalar.activation(lnf[:], sa["eo"][:, :, 0], AF.Ln)

            wsum = sm.tile([P, W, 19], f32, tag="wsum")
            nc.vector.scalar_tensor_tensor(
                out=wsum[:], in0=lns[:], scalar=0.0, in1=ba[:],
                op0=A.bypass, op1=A.subtract)
            rsub = sm.tile([P, W], f32, tag="rsub")
            nc.vector.tensor_reduce(out=rsub[:], in_=wsum[:], axis=AX.X,
                                    op=A.add)
            rmain = sm.tile([P, W], f32, tag="rmain")
            nc.vector.scalar_tensor_tensor(
                out=rmain[:], in0=lnf[:], scalar=0.0, in1=q[:],
                op0=A.bypass, op1=A.subtract)
            rtot = sm.tile([P, W], f32, tag="rtot")
            nc.vector.scalar_tensor_tensor(
                out=rtot[:], in0=rsub[:], scalar=ALPHA, in1=rmain[:],
                op0=A.mult, op1=A.add)
            pt = sm.tile([P, 1], f32, tag="pt")
            nc.vector.tensor_reduce(out=pt[:], in_=rtot[:], axis=AX.X,
                                    op=A.add)
            nc.vector.scalar_tensor_tensor(
                out=core_acc[:], in0=core_acc[:], scalar=0.0, in1=pt[:],
                op0=A.bypass, op1=A.add)

        ones_t = pe.tile([P, 1], f32, tag="ones")
        nc.vector.memset(ones_t[:], 1.0)
        ps = ctx.enter_context(tc.tile_pool(name="ps", bufs=1, space="PSUM"))
        tot_ps = ps.tile([1, 1], f32, tag="tot")
        nc.tensor.matmul(tot_ps[:], ones_t[:], core_acc[:])
        total = pe.tile([1, 1], f32, tag="total")
        nc.scalar.copy(total[:], tot_ps[:])
        nc.sync.dma_start(out=res_d.ap(), in_=total[:])

    nc.compile()
    return nc


_PROGRAM_CACHE = {}


def _get_program(rows, W):
    key = (rows, W)
    if key not in _PROGRAM_CACHE:
        _PROGRAM_CACHE[key] = build_core_program(rows, W)
    return _PROGRAM_CACHE[key]


def kernel(output, targets):
    output = np.ascontiguousarray(np.asarray(output, dtype=np.float32))
    targets = np.ascontiguousarray(np.asarray(targets, dtype=np.float32))
    assert output.shape == (B, C) and targets.shape == (B, C)

    from concourse.bass_utils import run_bass_kernel_spmd

    nc = _get_program(B_CORE, 16)
    in_maps = []
    for ci in range(N_CORES):
        lo, hi = ci * B_CORE, (ci + 1) * B_CORE
        in_maps.append({"targets": targets[lo:hi], "output": output[lo:hi]})
    res = run_bass_kernel_spmd(nc, in_maps, list(range(N_CORES)))
    partials = [float(res.results[ci]["out"].reshape(-1)[0])
                for ci in range(N_CORES)]
    total = float(np.sum(np.asarray(partials, dtype=np.float64)))
    return np.float32(total / B)
